# revision 1
# baseline (speedup 1.0000x reference)
"""Trainium2 Bass kernel for relative-position attention (nn_Attention).

Reference computation (B=16, C=128, H=W=32, HEADS=4, d=32, N=1024):
    qkv  = W_qkv @ x                          (1x1 conv, per-pixel matmul)
    S    = scale * (q^T k + q^T r)            where r = rw + rh  (broadcast)
         = scale * q^T (k + r)                <- position term folds into k
    P    = softmax(S, axis=-1)
    out  = P @ v^T

Sharding: data-parallel over batch, 2 batches per core on 8 cores.
W_qkv / rw / rh replicated. No collectives.

Per-core kernel layout (all on-chip, nothing round-trips to DRAM):
  - qkv projection: wT [128c, 384o] stationary, x_b [128c, 1024] moving.
  - S^T per (head, j-chunk): lhsT = (k+r)_h [32d, 128j], rhs = q_h [32d, 1024i]
    -> PSUM [128j, 1024i]; exp via ScalarE (no max-subtraction needed:
    logits are O(10), far below fp32 exp overflow).
  - O  per head: lhsT = [v^T | 1] [128j, 33], rhs = E [128j, 1024i],
    accumulated over 8 j-chunks -> PSUM [33, 1024]; row 32 is the softmax
    denominator Z (ones column trick shares the matmul stream).
  - normalize (entirely off the PE): Z row -> SBUF, DMA-repartitioned to
    [32,32] so the DVE reciprocal runs on 32 lanes instead of 1, bounced
    through a DRAM row and read back with a step-0 partition-broadcast AP,
    then out = O * (1/Z) on VectorE.

The previous head's O matmuls are interleaved between the current head's S
chunks so the PE (the bottleneck engine, saturated ~96-100%) streams
continuously instead of pacing itself to ScalarE's exp drain rate.

Matmuls run in float32r by default (reduced-mantissa fp32; measured 3.7e-4
output rel err vs 4e-3 for bf16, at ~15% lower speed). fp32r operands must
be produced rounded, so every tile feeding a matmul is written as float32r
by the producing engine op; q/k+r/E/vt get that for free from the copies
they already need, x and W each pay one extra rounding copy.

q and k+r are stored as head-pair tiles of 64 partitions because PE matmul
operands must base at partition 0/32/64 (quadrant 3 is unusable).
"""

import numpy as np

B, C, H, W = 16, 128, 32, 32
HEADS = 4
D = C // HEADS          # 32
N = H * W               # 1024
SCALE = float(D) ** -0.5
NCORES = 8
BPC = B // NCORES       # batches per core

# matmul input dtypes per stage; "f32r" = reduced-mantissa fp32 (measured
# ~3.7e-4 output rel err, within ~15% of bf16 speed on this part since the
# PE streams at 1.2 GHz either way), "bf16" = fastest but ~4e-3 rel err,
# "f32" = exact but 4 cycles/row. QKV_DTYPE covers the projection, S_DTYPE
# the q/(k+r) score matmul, O_DTYPE the E/v attention-apply matmul.
import os as _os
QKV_DTYPE = _os.environ.get("KQKV_DT", "f32r")
S_DTYPE = _os.environ.get("KS_DT", "f32r")
O_DTYPE = _os.environ.get("KO_DT", "f32r")


def _build_kernel(nc, tc, tile, mybir, x_ap, wT_ap, rw_ap, rh_ap, out_ap):
    import concourse.bass as bass
    from concourse.masks import make_identity

    f32 = mybir.dt.float32
    DT = {"f32r": mybir.dt.float32r, "f32": mybir.dt.float32,
          "bf16": mybir.dt.bfloat16}
    mdt = DT[QKV_DTYPE]
    sdt = DT[S_DTYPE]
    odt = DT[O_DTYPE]

    const = tc.alloc_tile_pool(name="const", bufs=1)
    sb = tc.alloc_tile_pool(name="sb", bufs=2)
    epool = tc.alloc_tile_pool(name="epool", bufs=20)
    vt1pool = tc.alloc_tile_pool(name="vt1pool", bufs=16)
    psmm = tc.alloc_tile_pool(name="psmm", bufs=2, space="PSUM")
    psacc = tc.alloc_tile_pool(name="psacc", bufs=2, space="PSUM")
    dscratch = tc.alloc_tile_pool(name="dscratch", bufs=4, space="DRAM")

    # --- constants / replicated inputs ---
    identity = const.tile([128, 128], f32)
    make_identity(nc, identity[:])
    ones_f = const.tile([128, 32], f32)
    nc.vector.memset(ones_f[:], 1.0)
    # prefetch batch 0's x before the (smaller) weight DMAs so the first
    # qkv matmul's moving operand is ready sooner
    x0_s = sb.tile([128, N], f32, tag="x", name="x0_s")
    for nf in range(2):
        nc.sync.dma_start(out=x0_s[:, nf * 512:(nf + 1) * 512],
                          in_=x_ap[0, :, nf * 512:(nf + 1) * 512])
    w_s = const.tile([128, 3 * C], f32)
    nc.sync.dma_start(out=w_s[:], in_=wT_ap[:])
    rw_s = const.tile([128, W], f32)
    nc.sync.dma_start(out=rw_s[:], in_=rw_ap[:])
    rh_s = const.tile([128, H], f32)
    nc.sync.dma_start(out=rh_s[:], in_=rh_ap[:])

    if mdt != f32:
        w_r = const.tile([128, 3 * C], mdt)
        nc.vector.tensor_copy(out=w_r[:], in_=w_s[:])
    else:
        w_r = w_s

    # r[p, y*W + x] = rw[p, x] + rh[p, y] in one DVE op via step-0 free dims
    r_s = const.tile([128, N], f32)
    rw_b = bass.AP(tensor=rw_s.tensor, offset=rw_s.offset,
                   ap=[list(rw_s.ap[0]), [0, H], list(rw_s.ap[1])])
    rh_b = bass.AP(tensor=rh_s.tensor, offset=rh_s.offset,
                   ap=[list(rh_s.ap[0]), list(rh_s.ap[1]), [0, W]])
    nc.vector.tensor_add(
        out=r_s[:].rearrange("p (y x) -> p y x", y=H), in0=rh_b, in1=rw_b
    )

    # Software pipelining: the previous head's 16 O matmuls are interleaved
    # between the current head's S chunks (2 O chunks after each S chunk), so
    # the PE streams continuously instead of pacing itself to ScalarE's exp
    # drain rate (2 PSUM score slots). A continuously-busy PE also keeps the
    # HAM clock-gate warm (2.4 GHz vs 1.2 GHz). The previous head's
    # normalize runs entirely off the PE (DVE reciprocal + DMA broadcast)
    # and is spliced in one head later still.
    prev_o = []       # pending O-matmul thunks for the previous head
    pending = []      # pending normalize tails

    def emit_pending():
        while pending:
            pending.pop(0)()

    def finish_head(h, b, ps_o, out_s):
        """Emit after head h's O matmuls: off-PE normalize chain.
        Z row -> SBUF, DMA-repartition [1,1024]->[32,32] so the reciprocal
        runs on 32 DVE lanes instead of 1, DMA to a DRAM bounce row, read it
        back partition-broadcast, then out = O * R."""
        z_c = sb.tile([1, N], f32, tag="zc", name=f"zc{h}")
        nc.scalar.copy(out=z_c[:], in_=ps_o[D:D + 1, :])
        z32 = sb.tile([D, H], f32, tag="z32", name=f"z32_{h}")
        nc.sync.dma_start(out=z32[:], in_=z_c[:])
        rz32 = sb.tile([D, H], f32, tag="rz32", name=f"rz32_{h}")
        nc.vector.reciprocal(out=rz32[:], in_=z32[:])
        r_d = dscratch.tile([1, N], f32, tag="rd", name=f"rd{h}")
        nc.sync.dma_start(out=r_d[:], in_=rz32[:])
        # issue the broadcast read eagerly so its ~2us completion-semaphore
        # latency overlaps other work instead of stalling the deferred mul
        rb = sb.tile([D, N], f32, tag="rb")
        nc.sync.dma_start(out=rb[:], in_=r_d[0, :].partition_broadcast(D))

        def norm_tail():
            nc.vector.tensor_mul(
                out=out_s[h * D:(h + 1) * D, :], in0=ps_o[0:D, :], in1=rb[:]
            )
            # per-head output DMA so the kernel tail only waits on the last
            # head's 128KB slice, not the whole batch
            nc.sync.dma_start(
                out=out_ap[b, h * D:(h + 1) * D, :],
                in_=out_s[h * D:(h + 1) * D, :],
            )

        pending.append(norm_tail)

    for b in range(BPC):
        # load + round x in halves so the first qkv matmul starts sooner
        # (batch 0's x was prefetched above, before the weight DMAs)
        if b == 0:
            x_s = x0_s
        else:
            x_s = sb.tile([128, N], f32, tag="x", name=f"x{b}_s")
        x_r = x_s
        if mdt != f32:
            x_r = sb.tile([128, N], mdt, tag="xr", name="x_r")
        for nf in range(2):
            sl = slice(nf * 512, (nf + 1) * 512)
            if b > 0:
                nc.sync.dma_start(out=x_s[:, sl], in_=x_ap[b, :, sl])
            if mdt != f32:
                nc.gpsimd.tensor_copy(out=x_r[:, sl], in_=x_s[:, sl])

        # --- qkv projection: psum rows m*128.. are q/k/v, each [128(h,d), N] ---
        q_p = [sb.tile([64, N], sdt, tag=f"q{i}", name=f"q{i}") for i in range(2)]
        kp_p = [sb.tile([64, N], sdt, tag=f"kp{i}", name=f"kp{i}") for i in range(2)]
        v_s = sb.tile([128, N], f32, tag="v")
        # v first: the PE transposes depend only on v, so they can fill the
        # pipeline while the q/k+r copies for the S matmuls drain
        for m in (2, 1, 0):
            ps = psmm.tile([128, N], f32, tag="mm", name=f"ps_qkv{m}")
            for nf in range(2):
                nc.tensor.matmul(
                    ps[:, nf * 512:(nf + 1) * 512],
                    lhsT=w_r[:, m * 128:(m + 1) * 128],
                    rhs=x_r[:, nf * 512:(nf + 1) * 512],
                    start=True, stop=True,
                )
            if m == 0:
                # 1/sqrt(d) score scale is folded into W_qkv's q rows on host
                for i in range(2):
                    nc.scalar.activation(
                        out=q_p[i][:], in_=ps[i * 64:(i + 1) * 64, :],
                        func=mybir.ActivationFunctionType.Copy, scale=1.0,
                    )
            elif m == 1:
                for i in range(2):
                    nc.vector.tensor_add(
                        out=kp_p[i][:], in0=ps[i * 64:(i + 1) * 64, :],
                        in1=r_s[i * 64:(i + 1) * 64, :],
                    )
            else:
                nc.vector.tensor_copy(out=v_s[:], in_=ps[:])

        # --- v^T tiles with ones column: vt1[jc][:, h, :] = [v_h^T | 1] ---
        vt1 = []
        for jc in range(8):
            ps_t = psmm.tile([128, 128], f32, tag="mm", name=f"ps_t{jc}")
            nc.tensor.transpose(ps_t[:], v_s[:, jc * 128:(jc + 1) * 128], identity[:])
            vt = vt1pool.tile([128, HEADS, D + 1], odt, tag="vt1", name=f"vt{jc}")
            nc.vector.tensor_copy(
                out=vt[:, :, D:D + 1],
                in_=ones_f[:, 0:HEADS].rearrange("p (h o) -> p h o", o=1),
            )
            nc.vector.tensor_copy(
                out=vt[:, :, 0:D],
                in_=ps_t[:].rearrange("p (h d) -> p h d", h=HEADS),
            )
            vt1.append(vt)

        out_s = sb.tile([128, N], f32, tag="out")

        # --- attention, software-pipelined across heads ---
        for h in range(4):
            lo = (h % 2) * D
            q_h = q_p[h // 2][lo:lo + D, :]
            kp_h = kp_p[h // 2][lo:lo + D, :]
            last_head = (b == BPC - 1 and h == HEADS - 1)
            own_o = []

            e_tiles = []
            if last_head:
                ps_o_pre = psacc.tile([D + 1, N], f32, tag="acc",
                                      name=f"ps_o{h}")

                def o_chunk_pre(jc, ps_o=ps_o_pre, e_tiles=e_tiles, vt1=vt1,
                                h=h, b=b, out_s=out_s):
                    for nf in range(2):
                        nc.tensor.matmul(
                            ps_o[:, nf * 512:(nf + 1) * 512],
                            lhsT=vt1[jc][:, h, :],
                            rhs=e_tiles[jc][:, nf * 512:(nf + 1) * 512],
                            start=(jc == 0), stop=(jc == 7),
                        )
                    if jc == 7:
                        finish_head(h, b, ps_o, out_s)

                own_o = [lambda jc=jc: o_chunk_pre(jc) for jc in range(8)]
            for jc in range(8):
                ps_s = psmm.tile([128, N], f32, tag="mm", name=f"ps_s{h}_{jc}")
                if S_DTYPE == "bf16":
                    nc.tensor.matmul(
                        ps_s[:, :], lhsT=kp_h[:, jc * 128:(jc + 1) * 128],
                        rhs=q_h[:, :], start=True, stop=True,
                    )
                else:
                    for nf in range(2):
                        nc.tensor.matmul(
                            ps_s[:, nf * 512:(nf + 1) * 512],
                            lhsT=kp_h[:, jc * 128:(jc + 1) * 128],
                            rhs=q_h[:, nf * 512:(nf + 1) * 512],
                            start=True, stop=True,
                        )
                e_t = epool.tile([128, N], odt, tag="e", name=f"e{h}_{jc}")
                nc.scalar.activation(
                    out=e_t[:], in_=ps_s[:], func=mybir.ActivationFunctionType.Exp
                )
                e_tiles.append(e_t)
                # splice in one of the previous head's O chunks per S chunk
                # (8 S chunks, 8 O chunks -> evenly interleaved)
                if prev_o:
                    prev_o.pop(0)()
                # the very last head has no following S phase to hide its O
                # matmuls in, so run them inline right behind each exp
                if last_head and own_o:
                    own_o.pop(0)()
            while own_o:
                own_o.pop(0)()
            emit_pending()

            if last_head:
                continue
            # this head's O matmuls, deferred into the next head's S phase:
            # O = [v^T|1]^T E accumulated over j-chunks -> [33, N]; row 32 = Z
            ps_o = psacc.tile([D + 1, N], f32, tag="acc", name=f"ps_o{h}")

            def o_chunk(jc, ps_o=ps_o, e_tiles=e_tiles, vt1=vt1, h=h, b=b,
                        out_s=out_s):
                if O_DTYPE == "bf16":
                    nc.tensor.matmul(
                        ps_o[:, :], lhsT=vt1[jc][:, h, :],
                        rhs=e_tiles[jc][:, :],
                        start=(jc == 0), stop=(jc == 7),
                    )
                else:
                    for nf in range(2):
                        nc.tensor.matmul(
                            ps_o[:, nf * 512:(nf + 1) * 512],
                            lhsT=vt1[jc][:, h, :],
                            rhs=e_tiles[jc][:, nf * 512:(nf + 1) * 512],
                            start=(jc == 0), stop=(jc == 7),
                        )
                if jc == 7:
                    finish_head(h, b, ps_o, out_s)

            prev_o.extend([lambda jc=jc: o_chunk(jc) for jc in range(8)])

    # drain: last head's O matmuls and remaining normalizes
    while prev_o:
        prev_o.pop(0)()
    emit_pending()

    for p in (dscratch, psacc, psmm, vt1pool, epool, sb, const):
        p.release()


def build_nc():
    """Build the Bass module (shared by kernel() and test harnesses)."""
    import concourse.bacc as bacc
    import concourse.tile as tile
    from concourse import mybir

    f32 = mybir.dt.float32
    nc = bacc.Bacc("TRN2", target_bir_lowering=False, debug=False,
                   num_devices=NCORES)
    x_ap = nc.dram_tensor("x", [BPC, C, N], f32, kind="ExternalInput").ap()
    wT_ap = nc.dram_tensor("wT", [C, 3 * C], f32, kind="ExternalInput").ap()
    rw_ap = nc.dram_tensor("rw2", [HEADS * D, W], f32, kind="ExternalInput").ap()
    rh_ap = nc.dram_tensor("rh2", [HEADS * D, H], f32, kind="ExternalInput").ap()
    out_ap = nc.dram_tensor("out", [BPC, C, N], f32, kind="ExternalOutput").ap()

    with tile.TileContext(nc) as tc:
        _build_kernel(nc, tc, tile, mybir, x_ap, wT_ap, rw_ap, rh_ap, out_ap)
    nc.compile()
    return nc


def make_in_maps(x, W_qkv, rw, rh):
    x_ = np.ascontiguousarray(np.asarray(x, np.float32).reshape(B, C, N))
    wT = np.ascontiguousarray(np.asarray(W_qkv, np.float32).T)
    wT[:, 0:C] *= SCALE    # fold the attention score scale into q projection
    rw_ = np.ascontiguousarray(np.asarray(rw, np.float32).reshape(HEADS * D, W))
    rh_ = np.ascontiguousarray(np.asarray(rh, np.float32).reshape(HEADS * D, H))
    return [
        {"x": x_[i * BPC:(i + 1) * BPC], "wT": wT, "rw2": rw_, "rh2": rh_}
        for i in range(NCORES)
    ]


def kernel(x, W_qkv, rw, rh):
    from concourse.bass_utils import run_bass_kernel_spmd

    nc = build_nc()
    in_maps = make_in_maps(x, W_qkv, rw, rh)
    res = None
    for attempt in range(3):
        try:
            res = run_bass_kernel_spmd(nc, in_maps, list(range(NCORES)))
            break
        except Exception:
            # transient device errors (e.g. NRT_EXEC_UNIT_UNRECOVERABLE after
            # an earlier crashed run) usually clear on retry
            if attempt == 2:
                raise
    out = np.concatenate([r["out"] for r in res.results], axis=0)
    return out.reshape(B, C, H, W).astype(np.float32)



# revision 26
# speedup vs baseline: 1.1039x; 1.1039x over previous
"""Trainium2 Bass kernel for relative-position attention (nn_Attention).

Reference computation (B=16, C=128, H=W=32, HEADS=4, d=32, N=1024):
    qkv  = W_qkv @ x                          (1x1 conv, per-pixel matmul)
    S    = scale * (q^T k + q^T r)            where r = rw + rh  (broadcast)
         = scale * q^T (k + r)                <- position term folds into k
    P    = softmax(S, axis=-1)
    out  = P @ v^T
Sharding: data-parallel over batch, 2 batches per core on 8 cores.

Design (v2): the kernel is ScalarE-bound -- exp of the full [N,N] score
matrix per (batch, head) is 8.4M elements/core and exp runs ONLY on the
ACT engine at 1 elem/cycle/lane. Everything else is organized to hide
under the exp stream:

  - S^T chunks are 4x ROW-TILED on the PE (K=d=32 -> four 32-row tiles,
    one per head, run concurrently) so PE time is insensitive to HAM
    clock state.
  - O = [v^T|1]^T E is 2x COL-TILED (M=33 -> two 64-col groups, head
    pairs), halving O wall time and fitting both pair accumulators in
    4 PSUM banks.
  - Per round (jc, nf): 4 S matmuls -> 2 psum tiles sA (heads 0,1) and
    sB (heads 2,3); exp as TWO activates so the next round's first two
    S tiles + prev round's O run during the second activate: the ACT
    engine streams gaplessly.
  - v^T is computed directly (x-chunk stationary x W_v moving), no PE
    transposes.
  - ScalarE does NOTHING but exp (table pre-loaded via a warmup
    activate at kernel start). All PSUM evacuation is on the DVE.
  - Normalize per head-pair: one strided Z copy, DMA repartition so the
    reciprocal runs on 64 lanes, DRAM-bounce broadcast back, one fused
    [97,1024] multiply, per-head output DMA.
"""

import numpy as np

B, C, H, W = 16, 128, 32, 32
HEADS = 4
D = C // HEADS          # 32
N = H * W               # 1024
SCALE = float(D) ** -0.5
NCORES = 8
BPC = B // NCORES       # batches per core

import os as _os
KV = _os.environ.get("BASS_KV", "2")


def _build_kernel_v2(nc, tc, tile, mybir, x_ap, wT_ap, rw_ap, rh_ap, out_ap,
                     dbg=None):
    import concourse.bass as bass

    f32 = mybir.dt.float32
    f32r = mybir.dt.float32r
    EXPF = mybir.ActivationFunctionType.Exp

    const = tc.alloc_tile_pool(name="const", bufs=1)
    xpool = tc.alloc_tile_pool(name="xpool", bufs=2)
    qkpool = tc.alloc_tile_pool(name="qkpool", bufs=2)
    epool = tc.alloc_tile_pool(name="epool", bufs=6)
    vtpool = tc.alloc_tile_pool(name="vtpool", bufs=2)
    zpool = tc.alloc_tile_pool(name="zpool", bufs=2)
    ospool = tc.alloc_tile_pool(name="ospool", bufs=2)
    psS = tc.alloc_tile_pool(name="psS", bufs=2, space="PSUM")
    psO = tc.alloc_tile_pool(name="psO", bufs=2, space="PSUM")
    dscratch = tc.alloc_tile_pool(name="dscratch", bufs=2, space="DRAM")

    # --- warmup: load the exp table set while the first DMAs run ---
    warm = const.tile([1, 8], f32)
    nc.vector.memset(warm[:], 0.5)
    warm2 = const.tile([1, 8], f32)
    nc.scalar.activation(out=warm2[:], in_=warm[:], func=EXPF)

    # --- constants / replicated inputs ---
    # prefetch batch 0's x before the (smaller) weight DMAs
    x_bufs = []
    for b in range(BPC):
        xb = xpool.tile([128, N], f32, tag=f"x{b}", name=f"x{b}")
        x_bufs.append(xb)
    for half in range(2):
        nc.sync.dma_start(out=x_bufs[0][:, half * 512:(half + 1) * 512],
                          in_=x_ap[0, :, half * 512:(half + 1) * 512])
    w_s = const.tile([128, 3 * C], f32)
    nc.sync.dma_start(out=w_s[:], in_=wT_ap[:])
    rw_s = const.tile([128, W], f32)
    nc.sync.dma_start(out=rw_s[:], in_=rw_ap[:])
    rh_s = const.tile([128, H], f32)
    nc.sync.dma_start(out=rh_s[:], in_=rh_ap[:])
    for b in range(1, BPC):
        for half in range(2):
            nc.sync.dma_start(out=x_bufs[b][:, half * 512:(half + 1) * 512],
                              in_=x_ap[b, :, half * 512:(half + 1) * 512])

    w_r = const.tile([128, 3 * C], f32r)
    nc.vector.tensor_copy(out=w_r[:], in_=w_s[:])

    # r[p, y*W + x] = rw[p, x] + rh[p, y] in one DVE op via step-0 free dims
    r_s = const.tile([128, N], f32)
    rw_b = bass.AP(tensor=rw_s.tensor, offset=rw_s.offset,
                   ap=[list(rw_s.ap[0]), [0, H], list(rw_s.ap[1])])
    rh_b = bass.AP(tensor=rh_s.tensor, offset=rh_s.offset,
                   ap=[list(rh_s.ap[0]), list(rh_s.ap[1]), [0, W]])
    nc.vector.tensor_add(
        out=r_s[:].rearrange("p (y x) -> p y x", y=H), in0=rh_b, in1=rw_b
    )

    # per-PAIR reciprocal-broadcast tiles; one tile per pair index so a
    # batch's deferred multiply is always emitted before the next batch's
    # broadcast overwrites it (Tile deps are emission-ordered).
    rb_tiles = [const.tile([128, N], f32, name=f"rb{p}") for p in range(2)]
    ones_f = const.tile([128, 32], f32)
    nc.vector.memset(ones_f[:], 1.0)
    zeros_f = const.tile([128, 64], f32)
    nc.vector.memset(zeros_f[:], 0.0)


    # x rounded to f32r on the DVE (2x_2P single-src SBUF copies)
    xr_bufs = []
    for b in range(BPC):
        xr = xpool.tile([128, N], f32r, tag=f"xr{b}", name=f"xr{b}")
        xr_bufs.append(xr)
    for half in range(2):
        sl = slice(half * 512, (half + 1) * 512)
        nc.vector.tensor_copy(out=xr_bufs[0][:, sl], in_=x_bufs[0][:, sl])

    # ---------------- per-batch state ----------------
    def phase_a_qk(b, q_all, kp_all):
        """qkv q/k projection + evacuation, emitted in nf halves so the
        first S round unblocks as soon as half 0 is evacuated."""
        xr = xr_bufs[b]
        ps_q = psS.tile([128, N], f32, tag="s", name=f"ps_q{b}")
        ps_k = psS.tile([128, N], f32, tag="s", name=f"ps_k{b}")
        for half in range(2):
            sl = slice(half * 512, (half + 1) * 512)
            nc.tensor.matmul(ps_q[:, sl], lhsT=w_r[:, 0:128], rhs=xr[:, sl],
                             start=True, stop=True)
            nc.tensor.matmul(ps_k[:, sl], lhsT=w_r[:, 128:256], rhs=xr[:, sl],
                             start=True, stop=True)
            nc.vector.tensor_copy(out=q_all[:, sl], in_=ps_q[:, sl])
            nc.vector.tensor_add(out=kp_all[:, sl], in0=ps_k[:, sl],
                                 in1=r_s[:, sl])

    def phase_a_v(b, vt_all):
        """v^T computed directly: x chunk stationary, W_v moving. Split
        across BOTH psS buffers (two allocations) so the pool's A/B parity
        is preserved and nothing here couples to the psO normalize path."""
        xr = xr_bufs[b]
        # O stationary layout: [128j, jc, h, 128m]; head h holds
        # [1 | v_h^T] at m-columns [64*(h%2), 64*(h%2)+33), zeros
        # elsewhere. M=128 keeps the matmul on the standard full-array
        # path (col tiling at position 64 trips the quadrant-3 XBUS bug);
        # zero columns just accumulate zeros into the pad partitions.
        # f32r memset lowers to invalid ISA -> broadcast-AP copies instead.
        for col in range(2):
            zb = bass.AP(tensor=zeros_f.tensor, offset=zeros_f.offset,
                         ap=[list(zeros_f.ap[0]), [0, 8], [0, HEADS],
                             [1, 64]])
            nc.vector.tensor_copy(out=vt_all[:, :, :, 64 * col:64 * (col + 1)],
                                  in_=zb)
        for h in range(HEADS):
            nc.vector.tensor_copy(
                out=vt_all[:, :, h, 64 * (h % 2):64 * (h % 2) + 1],
                in_=ones_f[:, 0:8].rearrange("p (j o) -> p j o", o=1),
            )
        for half in range(2):
            ps_v = psS.tile([128, 512], f32, tag="s", name=f"ps_v{b}_{half}")
            # one accumulation group for the whole bank: start=True again
            # would re-mark the full 2KB zero region and wipe earlier chunks
            for j in range(4):
                jc = half * 4 + j
                nc.tensor.matmul(ps_v[:, j * 128:(j + 1) * 128],
                                 lhsT=xr[:, jc * 128:(jc + 1) * 128],
                                 rhs=w_r[:, 256:384],
                                 start=(j == 0), stop=(j == 3))
            # vt[p, jc, h, 64*(h%2)+1 : +33] = v^T chunk, via explicit APs
            for h in range(HEADS):
                c0 = 64 * (h % 2) + 1
                o_ap = bass.AP(
                    tensor=vt_all.tensor,
                    offset=vt_all.offset + half * 4 * HEADS * 128
                    + h * 128 + c0,
                    ap=[list(vt_all.ap[0]), [HEADS * 128, 4], [1, D]],
                )
                i_ap = bass.AP(
                    tensor=ps_v.tensor,
                    offset=ps_v.offset + h * D,
                    ap=[list(ps_v.ap[0]), [HEADS * D, 4], [1, D]],
                )
                nc.vector.tensor_copy(out=o_ap, in_=i_ap)

    # pending per-round O thunk lists and normalize thunks
    def make_o_thunks(b, e_tiles, vt_all, po, jc, nf):
        """O matmuls for round (jc, nf): standard M=128 matmuls; the two
        heads of a pair land in disjoint partition ranges of one
        accumulation group via the column placement of their stationary."""
        def run(pair):
            eA_or_B = e_tiles[pair]  # pair 0 -> heads 0,1 in eA; pair 1 -> eB
            for e in range(2):
                h = 2 * pair + e
                nc.tensor.matmul(
                    po[pair][:, nf * 512:(nf + 1) * 512],
                    lhsT=vt_all[:, jc, h, :],
                    rhs=eA_or_B[:, e * 512:(e + 1) * 512],
                    start=(jc == 0 and e == 0), stop=(jc == 7 and e == 1),
                )
        return run

    def normalize_pair(b, pair, po_p, os_p, z_on_scalar):
        """Z rows live at partitions 32 (head 2p) and 96 (head 2p+1) of
        po_p... now at partitions 0 (head 2p) and 64 (head 2p+1) with the
        ones column first. One wide copy grabs both; repartition to 64 lanes for the
        reciprocal; DRAM-bounce broadcast back; one fused multiply.
        The Z copy goes on ScalarE when it fits in the inter-batch bubble
        (keeps the DVE free for the next batch's PSUM evacuations)."""
        z66 = zpool.tile([65, N], f32, tag="z66", name=f"z66_{b}_{pair}")
        if z_on_scalar:
            nc.scalar.copy(out=z66[:], in_=po_p[0:65, :])
        else:
            nc.vector.tensor_copy(out=z66[:], in_=po_p[0:65, :])
        # repartition: zr[k, c] for k<32 <- z66[0, 32k+c]; k>=32 <- z66[64, ...]
        zr = zpool.tile([64, D], f32, tag="zr", name=f"zr_{b}_{pair}")
        nc.sync.dma_start(out=zr[0:32, :], in_=z66[0:1, :])
        nc.sync.dma_start(out=zr[32:64, :], in_=z66[64:65, :])
        rz = zpool.tile([64, D], f32, tag="rz", name=f"rz_{b}_{pair}")
        nc.vector.reciprocal(out=rz[:], in_=zr[:])
        r_d = dscratch.tile([2, N], f32, tag="rd", name=f"rd_{b}_{pair}")
        nc.sync.dma_start(out=r_d[0:1, :], in_=rz[0:32, :])
        nc.sync.dma_start(out=r_d[1:2, :], in_=rz[32:64, :])
        # broadcast into full 64-row blocks: rows 32:64 / 96:128 get
        # harmless duplicates, so no memset is needed for the wide multiply
        rb = rb_tiles[pair]
        nc.sync.dma_start(out=rb[0:64, :], in_=r_d[0, :].partition_broadcast(64))
        nc.sync.dma_start(out=rb[64:128, :],
                          in_=r_d[1, :].partition_broadcast(64))

        def tail():
            # O rows sit at 1:33 / 65:97 (ones column first); one
            # full-width mul, junk rows multiply harmlessly
            nc.vector.tensor_mul(out=os_p[:], in0=po_p[:], in1=rb[:])
            for e in range(2):
                h = 2 * pair + e
                nc.sync.dma_start(
                    out=out_ap[b, h * D:(h + 1) * D, :],
                    in_=os_p[64 * e + 1:64 * e + 1 + D, :],
                )
        return tail

    # ---------------- main schedule ----------------
    prev_o = []        # O thunks pending from the previous round
    norm_tails = []

    for b in range(BPC):
        if b > 0:
            for half in range(2):
                sl = slice(half * 512, (half + 1) * 512)
                nc.vector.tensor_copy(out=xr_bufs[b][:, sl],
                                      in_=x_bufs[b][:, sl])
        q_all = qkpool.tile([128, N], f32r, tag="q", name=f"q{b}")
        kp_all = qkpool.tile([128, N], f32r, tag="kp", name=f"kp{b}")
        if dbg is not None and b == 0:
            pass  # dumps emitted after producers below
        vt_all = vtpool.tile([128, 8, HEADS, 128], f32r, tag="vt",
                             name=f"vt{b}")
        phase_a_qk(b, q_all, kp_all)

        po = []  # filled at t == 0, after phase_a_v's psO allocation

        e_hist = {}
        for t in range(16):
            jc, nf = t // 2, t % 2
            sA = psS.tile([128, N], f32, tag="s", name=f"sA{b}_{t}")
            sB = psS.tile([128, N], f32, tag="s", name=f"sB{b}_{t}")
            for h in (0, 1):
                nc.tensor.matmul(
                    sA[:, (h % 2) * 512:((h % 2) + 1) * 512],
                    lhsT=kp_all[32 * h:32 * h + 32, jc * 128:(jc + 1) * 128],
                    rhs=q_all[32 * h:32 * h + 32, nf * 512:(nf + 1) * 512],
                    start=True, stop=True, tile_position=(32 * h, 0),
                )
            for h in (2, 3):
                nc.tensor.matmul(
                    sB[:, (h % 2) * 512:((h % 2) + 1) * 512],
                    lhsT=kp_all[32 * h:32 * h + 32, jc * 128:(jc + 1) * 128],
                    rhs=q_all[32 * h:32 * h + 32, nf * 512:(nf + 1) * 512],
                    start=True, stop=True, tile_position=(32 * h, 0),
                )
            # v^T projection + vt build at t==0 (PE runs it during the
            # first activates, in the psS buffers to stay off the psO
            # normalize path)
            if t == 0:
                phase_a_v(b, vt_all)
                # the previous batch's normalize tails MUST be emitted
                # before this batch's po allocation: Tile dependencies are
                # emission-ordered, and the tail's multiply reads the po
                # banks this allocation will reuse
                while norm_tails:
                    norm_tails.pop(0)()
                po.extend(psO.tile([128, N], f32, tag="o", name=f"po{b}_{p}")
                          for p in range(2))
            # previous rounds' O matmuls run during this round's activates;
            # for later batches hold them back until the previous batch's
            # normalize (which frees the po banks) has had time to land, so
            # a stalled O matmul can't sit in the PE queue ahead of this
            # round's S matmuls
            if b == 0 or t >= 3:
                for _ in range(4):
                    if prev_o:
                        prev_o.pop(0)()

            eA = epool.tile([128, N], f32r, tag="e", name=f"eA{b}_{t}")
            eB = epool.tile([128, N], f32r, tag="e", name=f"eB{b}_{t}")
            nc.scalar.activation(out=eA[:], in_=sA[:], func=EXPF)
            nc.scalar.activation(out=eB[:], in_=sB[:], func=EXPF)
            e_hist[t] = (eA, eB)
            if dbg is not None and b == 0 and t == 0:
                nc.sync.dma_start(out=dbg["q"], in_=q_all[:])
                nc.sync.dma_start(out=dbg["kp"], in_=kp_all[:])
                nc.sync.dma_start(out=dbg["eA0"], in_=eA[:])
                nc.sync.dma_start(out=dbg["eB0"], in_=eB[:])
            if dbg is not None and b == 0 and t == 1:
                nc.sync.dma_start(out=dbg["vt"], in_=vt_all[:].rearrange("p j h d -> p (j h d)"))

            run = make_o_thunks(b, e_hist[t], vt_all, po, jc, nf)
            prev_o.extend([lambda pair=p, run=run: run(pair) for p in range(2)])


        # flush the last round's O matmuls
        while prev_o:
            prev_o.pop(0)()

        os_tiles = [ospool.tile([128, N], f32, tag="os", name=f"os{b}_{p}")
                    for p in range(2)]
        last = b == BPC - 1
        for p in range(2):
            # pair 0's Z copy fits the inter-batch ScalarE bubble; pair 1's
            # goes on the DVE (mid-stream) except for the final batch where
            # ScalarE is free after the last exp
            norm_tails.append(normalize_pair(b, p, po[p], os_tiles[p],
                                             z_on_scalar=(p == 0 or last)))

    while norm_tails:
        norm_tails.pop(0)()

    for p in (dscratch, psO, psS, ospool, zpool, vtpool, epool,
              qkpool, xpool, const):
        p.release()


def _build_kernel_v1(nc, tc, tile, mybir, x_ap, wT_ap, rw_ap, rh_ap, out_ap):
    import kernel_v1_backup as kv1
    kv1._build_kernel(nc, tc, tile, mybir, x_ap, wT_ap, rw_ap, rh_ap, out_ap)


def build_nc():
    """Build the Bass module (shared by kernel() and test harnesses)."""
    import concourse.bacc as bacc
    import concourse.tile as tile
    from concourse import mybir

    f32 = mybir.dt.float32
    nc = bacc.Bacc("TRN2", target_bir_lowering=False, debug=False,
                   num_devices=NCORES)
    x_ap = nc.dram_tensor("x", [BPC, C, N], f32, kind="ExternalInput").ap()
    wT_ap = nc.dram_tensor("wT", [C, 3 * C], f32, kind="ExternalInput").ap()
    rw_ap = nc.dram_tensor("rw2", [HEADS * D, W], f32, kind="ExternalInput").ap()
    rh_ap = nc.dram_tensor("rh2", [HEADS * D, H], f32, kind="ExternalInput").ap()
    out_ap = nc.dram_tensor("out", [BPC, C, N], f32, kind="ExternalOutput").ap()

    dbg = None
    if _os.environ.get("KDBG"):
        f32r_ = mybir.dt.float32r
        dbg = {
            "q": nc.dram_tensor("dbg_q", [C, N], f32r_, kind="ExternalOutput").ap(),
            "kp": nc.dram_tensor("dbg_kp", [C, N], f32r_, kind="ExternalOutput").ap(),
            "eA0": nc.dram_tensor("dbg_eA0", [C, N], f32r_, kind="ExternalOutput").ap(),
            "eB0": nc.dram_tensor("dbg_eB0", [C, N], f32r_, kind="ExternalOutput").ap(),
            "vt": nc.dram_tensor("dbg_vt", [C, 8 * HEADS * 128], f32r_, kind="ExternalOutput").ap(),
        }
    with tile.TileContext(nc) as tc:
        if KV == "1":
            _build_kernel_v1(nc, tc, tile, mybir, x_ap, wT_ap, rw_ap, rh_ap,
                             out_ap)
        else:
            _build_kernel_v2(nc, tc, tile, mybir, x_ap, wT_ap, rw_ap, rh_ap,
                             out_ap, dbg=dbg)
    nc.compile()
    return nc


def make_in_maps(x, W_qkv, rw, rh):
    x_ = np.ascontiguousarray(np.asarray(x, np.float32).reshape(B, C, N))
    wT = np.ascontiguousarray(np.asarray(W_qkv, np.float32).T)
    wT[:, 0:C] *= SCALE    # fold the attention score scale into q projection
    rw_ = np.ascontiguousarray(np.asarray(rw, np.float32).reshape(HEADS * D, W))
    rh_ = np.ascontiguousarray(np.asarray(rh, np.float32).reshape(HEADS * D, H))
    return [
        {"x": x_[i * BPC:(i + 1) * BPC], "wT": wT, "rw2": rw_, "rh2": rh_}
        for i in range(NCORES)
    ]


def kernel(x, W_qkv, rw, rh):
    from concourse.bass_utils import run_bass_kernel_spmd

    nc = build_nc()
    in_maps = make_in_maps(x, W_qkv, rw, rh)
    res = None
    for attempt in range(3):
        try:
            res = run_bass_kernel_spmd(nc, in_maps, list(range(NCORES)))
            break
        except Exception:
            # transient device errors usually clear on retry
            if attempt == 2:
                raise
    out = np.concatenate([r["out"] for r in res.results], axis=0)
    return out.reshape(B, C, H, W).astype(np.float32)


# revision 27
# speedup vs baseline: 1.2439x; 1.1269x over previous
"""Trainium2 Bass kernel for relative-position attention (nn_Attention).

Reference computation (B=16, C=128, H=W=32, HEADS=4, d=32, N=1024):
    qkv  = W_qkv @ x                          (1x1 conv, per-pixel matmul)
    S    = scale * (q^T k + q^T r)            where r = rw + rh  (broadcast)
         = scale * q^T (k + r)                <- position term folds into k
    P    = softmax(S, axis=-1)
    out  = P @ v^T
Sharding: data-parallel over batch, 2 batches per core on 8 cores.

Design (v2): the kernel is ScalarE-bound -- exp of the full [N,N] score
matrix per (batch, head) is 8.4M elements/core and exp runs ONLY on the
ACT engine at 1 elem/cycle/lane. Everything else is organized to hide
under the exp stream:

  - S^T chunks are 4x ROW-TILED on the PE (K=d=32 -> four 32-row tiles,
    one per head, run concurrently) so PE time is insensitive to HAM
    clock state.
  - O = [v^T|1]^T E is 2x COL-TILED (M=33 -> two 64-col groups, head
    pairs), halving O wall time and fitting both pair accumulators in
    4 PSUM banks.
  - Per round (jc, nf): 4 S matmuls -> 2 psum tiles sA (heads 0,1) and
    sB (heads 2,3); exp as TWO activates so the next round's first two
    S tiles + prev round's O run during the second activate: the ACT
    engine streams gaplessly.
  - v^T is computed directly (x-chunk stationary x W_v moving), no PE
    transposes.
  - ScalarE does NOTHING but exp (table pre-loaded via a warmup
    activate at kernel start). All PSUM evacuation is on the DVE.
  - Normalize per head-pair: one strided Z copy, DMA repartition so the
    reciprocal runs on 64 lanes, DRAM-bounce broadcast back, one fused
    [97,1024] multiply, per-head output DMA.
"""

import numpy as np

B, C, H, W = 16, 128, 32, 32
HEADS = 4
D = C // HEADS          # 32
N = H * W               # 1024
SCALE = float(D) ** -0.5
NCORES = 8
BPC = B // NCORES       # batches per core

import os as _os
KV = _os.environ.get("BASS_KV", "2")


def _build_kernel_v2(nc, tc, tile, mybir, x_ap, wT_ap, rw_ap, rh_ap, out_ap,
                     dbg=None):
    import concourse.bass as bass

    f32 = mybir.dt.float32
    f32r = mybir.dt.bfloat16   # matmul operand dtype (bf16: 1 cycle/col
    # vs 2 for f32r, FWL-eligible ldweights; ~4e-3 end-to-end rel err)
    EXPF = mybir.ActivationFunctionType.Exp

    const = tc.alloc_tile_pool(name="const", bufs=1)
    xpool = tc.alloc_tile_pool(name="xpool", bufs=2)
    qkpool = tc.alloc_tile_pool(name="qkpool", bufs=2)
    epool = tc.alloc_tile_pool(name="epool", bufs=6)
    vtpool = tc.alloc_tile_pool(name="vtpool", bufs=2)
    zpool = tc.alloc_tile_pool(name="zpool", bufs=2)
    ospool = tc.alloc_tile_pool(name="ospool", bufs=2)
    psS = tc.alloc_tile_pool(name="psS", bufs=2, space="PSUM")
    psO = tc.alloc_tile_pool(name="psO", bufs=2, space="PSUM")
    dscratch = tc.alloc_tile_pool(name="dscratch", bufs=2, space="DRAM")

    # --- warmup: load the exp table set while the first DMAs run ---
    warm = const.tile([1, 8], f32)
    nc.vector.memset(warm[:], 0.5)
    warm2 = const.tile([1, 8], f32)
    nc.scalar.activation(out=warm2[:], in_=warm[:], func=EXPF)

    # --- constants / replicated inputs ---
    # prefetch batch 0's x before the (smaller) weight DMAs
    x_bufs = []
    for b in range(BPC):
        xb = xpool.tile([128, N], f32, tag=f"x{b}", name=f"x{b}")
        x_bufs.append(xb)
    for half in range(2):
        nc.sync.dma_start(out=x_bufs[0][:, half * 512:(half + 1) * 512],
                          in_=x_ap[0, :, half * 512:(half + 1) * 512])
    w_s = const.tile([128, 3 * C], f32)
    nc.sync.dma_start(out=w_s[:], in_=wT_ap[:])
    rw_s = const.tile([128, W], f32)
    nc.sync.dma_start(out=rw_s[:], in_=rw_ap[:])
    rh_s = const.tile([128, H], f32)
    nc.sync.dma_start(out=rh_s[:], in_=rh_ap[:])
    for b in range(1, BPC):
        for half in range(2):
            nc.sync.dma_start(out=x_bufs[b][:, half * 512:(half + 1) * 512],
                              in_=x_ap[b, :, half * 512:(half + 1) * 512])

    w_r = const.tile([128, 3 * C], f32r)
    nc.vector.tensor_copy(out=w_r[:], in_=w_s[:])

    # r[p, y*W + x] = rw[p, x] + rh[p, y] in one DVE op via step-0 free dims
    r_s = const.tile([128, N], f32)
    rw_b = bass.AP(tensor=rw_s.tensor, offset=rw_s.offset,
                   ap=[list(rw_s.ap[0]), [0, H], list(rw_s.ap[1])])
    rh_b = bass.AP(tensor=rh_s.tensor, offset=rh_s.offset,
                   ap=[list(rh_s.ap[0]), list(rh_s.ap[1]), [0, W]])
    nc.vector.tensor_add(
        out=r_s[:].rearrange("p (y x) -> p y x", y=H), in0=rh_b, in1=rw_b
    )

    # per-PAIR reciprocal-broadcast tiles; one tile per pair index so a
    # batch's deferred multiply is always emitted before the next batch's
    # broadcast overwrites it (Tile deps are emission-ordered).
    rb_tiles = [const.tile([128, N], f32, name=f"rb{p}") for p in range(2)]
    ones_f = const.tile([128, 32], f32)
    nc.vector.memset(ones_f[:], 1.0)
    zeros_f = const.tile([128, 64], f32)
    nc.vector.memset(zeros_f[:], 0.0)


    # O stationary tiles (one per batch), allocated up-front so the
    # never-changing zero/ones fills run once at startup instead of
    # clogging the DVE at each batch boundary. Layout: [128j, jc, h, 128m];
    # head h holds [1 | v_h^T] at m-columns [64*(h%2), 64*(h%2)+33), zeros
    # elsewhere. M=128 keeps the matmul on the standard full-array path
    # (col tiling at position 64 trips the quadrant-3 XBUS bug).
    vt_tiles = []
    for i in range(BPC):
        vt = vtpool.tile([128, 8, HEADS, 128], f32r, tag=f"vt{i}",
                         name=f"vt{i}")
        for col in range(2):
            zb = bass.AP(tensor=zeros_f.tensor, offset=zeros_f.offset,
                         ap=[list(zeros_f.ap[0]), [0, 8], [0, HEADS],
                             [1, 64]])
            nc.vector.tensor_copy(out=vt[:, :, :, 64 * col:64 * (col + 1)],
                                  in_=zb)
        for h in range(HEADS):
            nc.vector.tensor_copy(
                out=vt[:, :, h, 64 * (h % 2):64 * (h % 2) + 1],
                in_=ones_f[:, 0:8].rearrange("p (j o) -> p j o", o=1),
            )
        vt_tiles.append(vt)

    # x rounded to matmul dtype on the DVE
    xr_bufs = []
    for b in range(BPC):
        xr = xpool.tile([128, N], f32r, tag=f"xr{b}", name=f"xr{b}")
        xr_bufs.append(xr)
    for bb in range(BPC):
        for half in range(2):
            sl = slice(half * 512, (half + 1) * 512)
            nc.vector.tensor_copy(out=xr_bufs[bb][:, sl],
                                  in_=x_bufs[bb][:, sl])

    # ---------------- per-batch state ----------------
    def phase_a_qk(b, q_all, kp_all):
        """qkv q/k projection + evacuation, emitted in nf halves so the
        first S round unblocks as soon as half 0 is evacuated."""
        xr = xr_bufs[b]
        ps_q = psS.tile([128, N], f32, tag="s", name=f"ps_q{b}")
        ps_k = psS.tile([128, N], f32, tag="s", name=f"ps_k{b}")
        for half in range(2):
            sl = slice(half * 512, (half + 1) * 512)
            nc.tensor.matmul(ps_q[:, sl], lhsT=w_r[:, 0:128], rhs=xr[:, sl],
                             start=True, stop=True)
            nc.tensor.matmul(ps_k[:, sl], lhsT=w_r[:, 128:256], rhs=xr[:, sl],
                             start=True, stop=True)
            nc.vector.tensor_copy(out=q_all[:, sl], in_=ps_q[:, sl])
            nc.vector.tensor_add(out=kp_all[:, sl], in0=ps_k[:, sl],
                                 in1=r_s[:, sl])

    def phase_a_v(b, vt_all):
        """v^T computed directly: x chunk stationary, W_v moving. Split
        across BOTH psS buffers (two allocations) so the pool's A/B parity
        is preserved and nothing here couples to the psO normalize path."""
        xr = xr_bufs[b]
        for half in range(2):
            ps_v = psS.tile([128, 512], f32, tag="s", name=f"ps_v{b}_{half}")
            # one accumulation group for the whole bank: start=True again
            # would re-mark the full 2KB zero region and wipe earlier chunks
            for j in range(4):
                jc = half * 4 + j
                nc.tensor.matmul(ps_v[:, j * 128:(j + 1) * 128],
                                 lhsT=xr[:, jc * 128:(jc + 1) * 128],
                                 rhs=w_r[:, 256:384],
                                 start=(j == 0), stop=(j == 3))
            # vt[p, jc, h, 64*(h%2)+1 : +33] = v^T chunk, via explicit APs
            for h in range(HEADS):
                c0 = 64 * (h % 2) + 1
                o_ap = bass.AP(
                    tensor=vt_all.tensor,
                    offset=vt_all.offset + half * 4 * HEADS * 128
                    + h * 128 + c0,
                    ap=[list(vt_all.ap[0]), [HEADS * 128, 4], [1, D]],
                )
                i_ap = bass.AP(
                    tensor=ps_v.tensor,
                    offset=ps_v.offset + h * D,
                    ap=[list(ps_v.ap[0]), [HEADS * D, 4], [1, D]],
                )
                nc.vector.tensor_copy(out=o_ap, in_=i_ap)

    # pending per-round O thunk lists and normalize thunks
    def make_o_thunks(b, e_tiles, vt_all, po, jc, nf):
        """O matmuls for round (jc, nf): standard M=128 matmuls; the two
        heads of a pair land in disjoint partition ranges of one
        accumulation group via the column placement of their stationary."""
        def run(pair):
            eA_or_B = e_tiles[pair]  # pair 0 -> heads 0,1 in eA; pair 1 -> eB
            for e in range(2):
                h = 2 * pair + e
                nc.tensor.matmul(
                    po[pair][:, nf * 512:(nf + 1) * 512],
                    lhsT=vt_all[:, jc, h, :],
                    rhs=eA_or_B[:, e * 512:(e + 1) * 512],
                    start=(jc == 0 and e == 0), stop=(jc == 7 and e == 1),
                )
        return run

    def normalize_pair(b, pair, po_p, os_p, z_on_scalar):
        """Z rows live at partitions 32 (head 2p) and 96 (head 2p+1) of
        po_p... now at partitions 0 (head 2p) and 64 (head 2p+1) with the
        ones column first. One wide copy grabs both; repartition to 64 lanes for the
        reciprocal; DRAM-bounce broadcast back; one fused multiply.
        The Z copy goes on ScalarE when it fits in the inter-batch bubble
        (keeps the DVE free for the next batch's PSUM evacuations)."""
        z66 = zpool.tile([65, N], f32, tag="z66", name=f"z66_{b}_{pair}")
        if z_on_scalar:
            nc.scalar.copy(out=z66[:], in_=po_p[0:65, :])
        else:
            nc.vector.tensor_copy(out=z66[:], in_=po_p[0:65, :])
        # repartition: zr[k, c] for k<32 <- z66[0, 32k+c]; k>=32 <- z66[64, ...]
        zr = zpool.tile([64, D], f32, tag="zr", name=f"zr_{b}_{pair}")
        nc.sync.dma_start(out=zr[0:32, :], in_=z66[0:1, :])
        nc.sync.dma_start(out=zr[32:64, :], in_=z66[64:65, :])
        rz = zpool.tile([64, D], f32, tag="rz", name=f"rz_{b}_{pair}")
        nc.vector.reciprocal(out=rz[:], in_=zr[:])
        r_d = dscratch.tile([2, N], f32, tag="rd", name=f"rd_{b}_{pair}")
        nc.sync.dma_start(out=r_d[0:1, :], in_=rz[0:32, :])
        nc.sync.dma_start(out=r_d[1:2, :], in_=rz[32:64, :])
        # broadcast into full 64-row blocks: rows 32:64 / 96:128 get
        # harmless duplicates, so no memset is needed for the wide multiply
        rb = rb_tiles[pair]
        nc.sync.dma_start(out=rb[0:64, :], in_=r_d[0, :].partition_broadcast(64))
        nc.sync.dma_start(out=rb[64:128, :],
                          in_=r_d[1, :].partition_broadcast(64))

        def tail():
            # O rows sit at 1:33 / 65:97 (ones column first); one
            # full-width mul, junk rows multiply harmlessly
            nc.vector.tensor_mul(out=os_p[:], in0=po_p[:], in1=rb[:])
            for e in range(2):
                h = 2 * pair + e
                nc.sync.dma_start(
                    out=out_ap[b, h * D:(h + 1) * D, :],
                    in_=os_p[64 * e + 1:64 * e + 1 + D, :],
                )
        return tail

    # ---------------- main schedule ----------------
    prev_o = []        # O thunks pending from the previous round
    norm_tails = []

    for b in range(BPC):
        q_all = qkpool.tile([128, N], f32r, tag="q", name=f"q{b}")
        kp_all = qkpool.tile([128, N], f32r, tag="kp", name=f"kp{b}")
        if dbg is not None and b == 0:
            pass  # dumps emitted after producers below
        vt_all = vt_tiles[b]
        phase_a_qk(b, q_all, kp_all)

        po = []  # filled at t == 0, after phase_a_v's psO allocation

        e_hist = {}
        for t in range(16):
            jc, nf = t // 2, t % 2
            sA = psS.tile([128, N], f32, tag="s", name=f"sA{b}_{t}")
            sB = psS.tile([128, N], f32, tag="s", name=f"sB{b}_{t}")
            for h in (0, 1):
                nc.tensor.matmul(
                    sA[:, (h % 2) * 512:((h % 2) + 1) * 512],
                    lhsT=kp_all[32 * h:32 * h + 32, jc * 128:(jc + 1) * 128],
                    rhs=q_all[32 * h:32 * h + 32, nf * 512:(nf + 1) * 512],
                    start=True, stop=True, tile_position=(32 * h, 0),
                )
            for h in (2, 3):
                nc.tensor.matmul(
                    sB[:, (h % 2) * 512:((h % 2) + 1) * 512],
                    lhsT=kp_all[32 * h:32 * h + 32, jc * 128:(jc + 1) * 128],
                    rhs=q_all[32 * h:32 * h + 32, nf * 512:(nf + 1) * 512],
                    start=True, stop=True, tile_position=(32 * h, 0),
                )
            # v^T projection + vt build at t==0 (PE runs it during the
            # first activates, in the psS buffers to stay off the psO
            # normalize path)
            if t == 0:
                phase_a_v(b, vt_all)
                # the previous batch's normalize tails MUST be emitted
                # before this batch's po allocation: Tile dependencies are
                # emission-ordered, and the tail's multiply reads the po
                # banks this allocation will reuse
                while norm_tails:
                    norm_tails.pop(0)()
                po.extend(psO.tile([128, N], f32, tag="o", name=f"po{b}_{p}")
                          for p in range(2))
            # previous rounds' O matmuls run during this round's activates;
            # for later batches hold them back until the previous batch's
            # normalize (which frees the po banks) has had time to land, so
            # a stalled O matmul can't sit in the PE queue ahead of this
            # round's S matmuls
            if b == 0 or t >= 3:
                for _ in range(4):
                    if prev_o:
                        prev_o.pop(0)()

            eA = epool.tile([128, N], f32r, tag="e", name=f"eA{b}_{t}")
            eB = epool.tile([128, N], f32r, tag="e", name=f"eB{b}_{t}")
            nc.scalar.activation(out=eA[:], in_=sA[:], func=EXPF)
            nc.scalar.activation(out=eB[:], in_=sB[:], func=EXPF)
            e_hist[t] = (eA, eB)
            if dbg is not None and b == 0 and t == 0:
                nc.sync.dma_start(out=dbg["q"], in_=q_all[:])
                nc.sync.dma_start(out=dbg["kp"], in_=kp_all[:])
                nc.sync.dma_start(out=dbg["eA0"], in_=eA[:])
                nc.sync.dma_start(out=dbg["eB0"], in_=eB[:])
            if dbg is not None and b == 0 and t == 1:
                nc.sync.dma_start(out=dbg["vt"], in_=vt_all[:].rearrange("p j h d -> p (j h d)"))

            run = make_o_thunks(b, e_hist[t], vt_all, po, jc, nf)
            prev_o.extend([lambda pair=p, run=run: run(pair) for p in range(2)])


        # flush the last round's O matmuls
        while prev_o:
            prev_o.pop(0)()

        os_tiles = [ospool.tile([128, N], f32, tag="os", name=f"os{b}_{p}")
                    for p in range(2)]
        last = b == BPC - 1
        for p in range(2):
            # pair 0's Z copy fits the inter-batch ScalarE bubble; pair 1's
            # goes on the DVE (mid-stream) except for the final batch where
            # ScalarE is free after the last exp
            norm_tails.append(normalize_pair(b, p, po[p], os_tiles[p],
                                             z_on_scalar=(p == 0 or last)))

    while norm_tails:
        norm_tails.pop(0)()

    for p in (dscratch, psO, psS, ospool, zpool, vtpool, epool,
              qkpool, xpool, const):
        p.release()


def _build_kernel_v1(nc, tc, tile, mybir, x_ap, wT_ap, rw_ap, rh_ap, out_ap):
    import kernel_v1_backup as kv1
    kv1._build_kernel(nc, tc, tile, mybir, x_ap, wT_ap, rw_ap, rh_ap, out_ap)


def build_nc():
    """Build the Bass module (shared by kernel() and test harnesses)."""
    import concourse.bacc as bacc
    import concourse.tile as tile
    from concourse import mybir

    f32 = mybir.dt.float32
    nc = bacc.Bacc("TRN2", target_bir_lowering=False, debug=False,
                   num_devices=NCORES)
    x_ap = nc.dram_tensor("x", [BPC, C, N], f32, kind="ExternalInput").ap()
    wT_ap = nc.dram_tensor("wT", [C, 3 * C], f32, kind="ExternalInput").ap()
    rw_ap = nc.dram_tensor("rw2", [HEADS * D, W], f32, kind="ExternalInput").ap()
    rh_ap = nc.dram_tensor("rh2", [HEADS * D, H], f32, kind="ExternalInput").ap()
    out_ap = nc.dram_tensor("out", [BPC, C, N], f32, kind="ExternalOutput").ap()

    dbg = None
    if _os.environ.get("KDBG"):
        f32r_ = mybir.dt.float32r
        dbg = {
            "q": nc.dram_tensor("dbg_q", [C, N], f32r_, kind="ExternalOutput").ap(),
            "kp": nc.dram_tensor("dbg_kp", [C, N], f32r_, kind="ExternalOutput").ap(),
            "eA0": nc.dram_tensor("dbg_eA0", [C, N], f32r_, kind="ExternalOutput").ap(),
            "eB0": nc.dram_tensor("dbg_eB0", [C, N], f32r_, kind="ExternalOutput").ap(),
            "vt": nc.dram_tensor("dbg_vt", [C, 8 * HEADS * 128], f32r_, kind="ExternalOutput").ap(),
        }
    with tile.TileContext(nc) as tc:
        if KV == "1":
            _build_kernel_v1(nc, tc, tile, mybir, x_ap, wT_ap, rw_ap, rh_ap,
                             out_ap)
        else:
            _build_kernel_v2(nc, tc, tile, mybir, x_ap, wT_ap, rw_ap, rh_ap,
                             out_ap, dbg=dbg)
    nc.compile()
    return nc


def make_in_maps(x, W_qkv, rw, rh):
    x_ = np.ascontiguousarray(np.asarray(x, np.float32).reshape(B, C, N))
    wT = np.ascontiguousarray(np.asarray(W_qkv, np.float32).T)
    wT[:, 0:C] *= SCALE    # fold the attention score scale into q projection
    rw_ = np.ascontiguousarray(np.asarray(rw, np.float32).reshape(HEADS * D, W))
    rh_ = np.ascontiguousarray(np.asarray(rh, np.float32).reshape(HEADS * D, H))
    return [
        {"x": x_[i * BPC:(i + 1) * BPC], "wT": wT, "rw2": rw_, "rh2": rh_}
        for i in range(NCORES)
    ]


def kernel(x, W_qkv, rw, rh):
    from concourse.bass_utils import run_bass_kernel_spmd

    nc = build_nc()
    in_maps = make_in_maps(x, W_qkv, rw, rh)
    res = None
    for attempt in range(3):
        try:
            res = run_bass_kernel_spmd(nc, in_maps, list(range(NCORES)))
            break
        except Exception:
            # transient device errors usually clear on retry
            if attempt == 2:
                raise
    out = np.concatenate([r["out"] for r in res.results], axis=0)
    return out.reshape(B, C, H, W).astype(np.float32)
